# revision 10
# baseline (speedup 1.0000x reference)
"""CLIP-style contrastive (HCL) loss for B=4096, f32 logits on 8 trn2 cores.

Math reduction (BETA=1, t=0.5, tau+=0.1):
  - imp == neg, so reweight_neg = sum(neg^2) * N / sum(neg).
  - Row i and row i+B of the 2Bx2B sim matrix hold identical value multisets
    (both are {row_i(L), col_i(L)} minus two copies of L[i,i]), so
    loss[i] == loss[i+B] and the mean over 2B rows == mean over B rows.
  - Everything reduces to row sums + col sums of E = exp(2L) and E2 = exp(4L),
    plus the diagonal of L.

Device work per core (rows k*512..(k+1)*512 of L, cast to bf16 on host):
  - 8 half-tiles [128, 2048]: ACT exp(2x)->bf16 E with fused fp32 row-sum,
    DVE tensor_tensor E*E->bf16 E2 (2x_1p mode) + tensor_scalar copy pass
    with fused fp32 row-sum (4x_2p mode),
    PE ones-matmul per-column sums into PSUM (E at partition 0, E2 at 32).
  - All 8 input DMAs issued up-front (lpool bufs=8) so HBM streams ahead
    of ACT; dummy exp before the loop hoists the ACT table load under DMA.
Host: assemble sums, per-row loss formula over 4096 rows in f64, mean.
"""

import os

import numpy as np
import ml_dtypes

import concourse.bacc as bacc
import concourse.bass as bass
import concourse.tile as tile
from concourse import mybir
from concourse.bass_utils import run_bass_kernel_spmd

B = 4096
N_CORES = 8
ROWS_PER_CORE = B // N_CORES  # 512
P = 128
TILES = ROWS_PER_CORE // P  # 4
CHUNK = 512  # matmul free-dim max (one PSUM bank)
HALF = B // 2  # 2048 cols per half-tile
NHALF = 2 * TILES  # 8 half-tiles per core

TAU_PLUS = 0.1
TEMPERATURE = 0.5
EPS = 1e-8

USE_BF16_IN = bool(int(os.environ.get("KERNEL_BF16_IN", "1")))
# "stt":   single scalar_tensor_tensor (1x rate) — square + fused rowsum in
#          one 2252ns pass; measured faster than any split (accum_out forces
#          1x on HW, so tensor_tensor(2x) + tensor_scalar-accum(1x) loses).
# "split": tensor_tensor square + tensor_scalar accum pass
SQ_MODE = os.environ.get("KERNEL_SQ", "stt")
USE_CSTAT = bool(int(os.environ.get("KERNEL_CSTAT", "1")))  # chunk-stationary colsums
LPOOL_BUFS = int(os.environ.get("KERNEL_LPOOL_BUFS", "8"))
EPOOL_BUFS = int(os.environ.get("KERNEL_EPOOL_BUFS", "4"))
WARM_ACT = bool(int(os.environ.get("KERNEL_WARM_ACT", "1")))

_NC = None
LAST_RESULTS = None  # BassKernelResults of the most recent run (for test harness)

# Work pieces per core: (row_tile, col_start, col_len). The last half-tile
# is split into two quarters so the tail of the ACT->DVE->PE pipeline drains
# in half-size steps (~1.1us shorter critical path).
PIECES = [(h // 2, (h % 2) * HALF, HALF) for h in range(NHALF - 1)] + [
    (TILES - 1, HALF, HALF // 2),
    (TILES - 1, HALF + HALF // 2, HALF // 2),
]
NPIECE = len(PIECES)  # 9


def _build_bass():
    in_dt = mybir.dt.bfloat16 if USE_BF16_IN else mybir.dt.float32
    edt = mybir.dt.bfloat16

    nc = bacc.Bacc(None)
    slab = nc.declare_dram_parameter("slab", [ROWS_PER_CORE, B], in_dt, isOutput=False)
    rowsums = nc.declare_dram_parameter(
        "rowsums", [P, 2 * NPIECE], mybir.dt.float32, isOutput=True
    )
    # Chunk-stationary layout: [128, 64] (E cols 0:32, E2 cols 32:64), where
    # colsum[m*128 + j] = out[j, m]. Row layout: [2, B].
    cshape = [P, 2 * (B // P)] if USE_CSTAT else [2, B]
    colsums = nc.declare_dram_parameter(
        "colsums", cshape, mybir.dt.float32, isOutput=True
    )

    with tile.TileContext(nc) as tc:
        with (
            tc.tile_pool(name="lpool", bufs=LPOOL_BUFS) as lpool,
            tc.tile_pool(name="epool", bufs=EPOOL_BUFS) as epool,
            tc.tile_pool(name="e2pool", bufs=EPOOL_BUFS) as e2pool,
            tc.tile_pool(name="singles", bufs=1) as singles,
            tc.tile_pool(name="psum", bufs=1, space="PSUM") as psum_pool,
        ):
            ones = nc.const_aps.tensor(1.0, (P, 1), mybir.dt.bfloat16)
            rs = singles.tile([P, 2 * NPIECE], mybir.dt.float32)
            junk = singles.tile([P, HALF], edt) if SQ_MODE == "split" else None
            warm = singles.tile([P, 1], mybir.dt.float32)
            # One PSUM bank per accumulator; output [128, 32] each.
            psE = psum_pool.tile([P, B // P], mybir.dt.float32)
            psE2 = psum_pool.tile([P, B // P], mybir.dt.float32)

            if WARM_ACT:
                # Loads the EXP activation table (~1.3us) while input DMAs run.
                nc.scalar.activation(
                    out=warm,
                    in_=ones,
                    func=mybir.ActivationFunctionType.Exp,
                    scale=2.0,
                )

            # Issue every input DMA up-front; lpool has one buf per piece, so
            # nothing gates them and HBM streams at full rate from t=0.
            # Descriptor generation (DIRECT2D) costs ~620ns of ISSUING-engine
            # time per dma_start, and a single dma_start only engages a few
            # of the 16 DMA queues (~110 GB/s), so: alternate issuance
            # between the two DMA-capable idle engines (sync, gpsimd), and
            # split piece 0 into four row-range dma_starts so its data (the
            # first thing ACT consumes) lands ~3.5us earlier.
            issuers = [nc.sync, nc.gpsimd]
            nissue = 0

            def issue(out_ap, in_ap):
                nonlocal nissue
                issuers[nissue % len(issuers)].dma_start(out=out_ap, in_=in_ap)
                nissue += 1

            ltiles = []
            for i, (t, c0, clen) in enumerate(PIECES):
                ltile = lpool.tile([P, clen], in_dt, tag=f"ltile{clen}")
                rows = slice(t * P, (t + 1) * P)
                if i == 0:
                    for r0 in range(0, P, P // 4):
                        rr = slice(r0, r0 + P // 4)
                        issue(
                            ltile[rr, :],
                            slab[t * P + r0 : t * P + r0 + P // 4, c0 : c0 + clen],
                        )
                else:
                    issue(ltile, slab[rows, c0 : c0 + clen])
                ltiles.append(ltile)

            for i, (t, c0, clen) in enumerate(PIECES):
                ltile = ltiles[i]
                etile = epool.tile([P, clen], edt, tag="etile")
                nc.scalar.activation(
                    out=etile,
                    in_=ltile,
                    func=mybir.ActivationFunctionType.Exp,
                    scale=2.0,
                    accum_out=rs[:, i : i + 1],
                )
                e2tile = e2pool.tile([P, clen], edt, tag="e2tile")
                if SQ_MODE == "split":
                    # E2 = E*E on DVE in 2x_1p mode, then a copy pass whose
                    # fused fp32 accumulator (4x_2p mode) yields the row-sum.
                    nc.vector.tensor_tensor(
                        out=e2tile, in0=etile, in1=etile, op=mybir.AluOpType.mult
                    )
                    nc.vector.tensor_scalar(
                        out=junk,
                        in0=e2tile,
                        scalar1=1.0,
                        scalar2=0.0,
                        op0=mybir.AluOpType.mult,
                        op1=mybir.AluOpType.add,
                        accum_out=rs[:, NPIECE + i : NPIECE + i + 1],
                    )
                else:
                    nc.vector.scalar_tensor_tensor(
                        out=e2tile,
                        in0=etile,
                        scalar=1.0,
                        in1=etile,
                        op0=mybir.AluOpType.mult,
                        op1=mybir.AluOpType.mult,
                        accum_out=rs[:, NPIECE + i : NPIECE + i + 1],
                    )

                # PSUM start_tensor_calc zeroes the whole 2KB (partition, bank)
                # zero-region lazily: only the FIRST matmul touching each psum
                # tensor may carry start=True; later writes to still-pending
                # bytes replace (i.e. add to zero), writes to touched bytes
                # accumulate. One start per tensor, ever.
                first = i == 0
                last = i == NPIECE - 1
                for m in range(clen // P):
                    gm = c0 // P + m
                    lsl = slice(m * P, (m + 1) * P)
                    nc.tensor.matmul(
                        psE[:, gm : gm + 1],
                        etile[:, lsl],
                        ones,
                        start=first and m == 0,
                        stop=last and m == clen // P - 1,
                        skip_group_check=True,
                    )
                for m in range(clen // P):
                    gm = c0 // P + m
                    lsl = slice(m * P, (m + 1) * P)
                    nc.tensor.matmul(
                        psE2[:, gm : gm + 1],
                        e2tile[:, lsl],
                        ones,
                        start=first and m == 0,
                        stop=last and m == clen // P - 1,
                        skip_group_check=True,
                    )

            # rowsums is ready at the last accum write; colsums needs the
            # PSUM eviction copies first. Issue the two output DMAs from
            # different engines so their descriptor generation overlaps.
            nc.sync.dma_start(out=rowsums[:, :], in_=rs)
            M = B // P  # 32
            cs = singles.tile([P, 2 * M], mybir.dt.float32)
            nc.vector.tensor_copy(cs[:, 0:M], psE)
            nc.scalar.copy(cs[:, M : 2 * M], psE2)
            nc.gpsimd.dma_start(out=colsums[:, :], in_=cs)
    # Bacc defers register allocation and sync-wait splitting to finalize();
    # run_bass_via_pjrt does not call it, so do it here.
    nc.finalize()
    return nc


def _get_nc():
    global _NC
    if _NC is None:
        _NC = _build_bass()
    return _NC


def kernel(logits: np.ndarray) -> np.ndarray:
    global LAST_RESULTS
    logits = np.ascontiguousarray(np.asarray(logits, dtype=np.float32))
    assert logits.shape == (B, B)

    nc = _get_nc()
    if USE_BF16_IN:
        cast = lambda a: np.ascontiguousarray(a.astype(ml_dtypes.bfloat16))
    else:
        cast = np.ascontiguousarray
    in_maps = [
        {"slab": cast(logits[k * ROWS_PER_CORE : (k + 1) * ROWS_PER_CORE, :])}
        for k in range(N_CORES)
    ]
    res = run_bass_kernel_spmd(
        nc,
        in_maps,
        core_ids=list(range(N_CORES)),
        trace=bool(int(os.environ.get("KERNEL_TRACE", "0"))),
    )
    LAST_RESULTS = res

    rowsum_E = np.empty(B, dtype=np.float64)
    rowsum_E2 = np.empty(B, dtype=np.float64)
    colsum_E = np.zeros(B, dtype=np.float64)
    colsum_E2 = np.zeros(B, dtype=np.float64)
    for k in range(N_CORES):
        r = res.results[k]
        rs = r["rowsums"].astype(np.float64)  # [128, 2*NPIECE]: piece partials
        sl = slice(k * ROWS_PER_CORE, (k + 1) * ROWS_PER_CORE)
        rsE = np.zeros((P, TILES))
        rsE2 = np.zeros((P, TILES))
        for i, (t, _, _) in enumerate(PIECES):
            rsE[:, t] += rs[:, i]
            rsE2[:, t] += rs[:, NPIECE + i]
        rowsum_E[sl] = rsE.T.reshape(-1)
        rowsum_E2[sl] = rsE2.T.reshape(-1)
        cssum = r["colsums"].astype(np.float64)
        if USE_CSTAT:
            M = B // P
            colsum_E += cssum[:, :M].T.reshape(-1)
            colsum_E2 += cssum[:, M:].T.reshape(-1)
        else:
            colsum_E += cssum[0]
            colsum_E2 += cssum[1]

    d = np.diagonal(logits)
    pos = np.exp(d.astype(np.float64) / TEMPERATURE)
    if USE_BF16_IN:
        # The device sums contain exp of the bf16-rounded diagonal; subtract
        # exactly what the device added.
        dD = d.astype(ml_dtypes.bfloat16).astype(np.float64)
    else:
        dD = d.astype(np.float64)
    posD = np.exp(dD / TEMPERATURE)
    N = 2 * B - 2
    S1 = rowsum_E + colsum_E - 2.0 * posD
    S2 = rowsum_E2 + colsum_E2 - 2.0 * posD * posD
    reweight = S2 * N / S1
    Ng = (-TAU_PLUS * N * pos + reweight) / (1.0 - TAU_PLUS)
    Ng = np.maximum(Ng, N * np.exp(-1.0 / TEMPERATURE))
    loss = -np.log(pos / (pos + Ng + EPS))
    return np.float32(loss.mean())


# revision 14
# speedup vs baseline: 1.0869x; 1.0869x over previous
"""CLIP-style contrastive (HCL) loss for B=4096, f32 logits on 8 trn2 cores.

Math reduction (BETA=1, t=0.5, tau+=0.1):
  - imp == neg, so reweight_neg = sum(neg^2) * N / sum(neg).
  - Row i and row i+B of the 2Bx2B sim matrix hold identical value multisets
    (both are {row_i(L), col_i(L)} minus two copies of L[i,i]), so
    loss[i] == loss[i+B] and the mean over 2B rows == mean over B rows.
  - Everything reduces to row sums + col sums of E = exp(2L) and E2 = exp(4L),
    plus the diagonal of L.

Device work per core (rows k*512..(k+1)*512 of L, cast to bf16 on host):
  - 8 half-tiles [128, 2048]: ACT exp(2x)->bf16 E with fused fp32 row-sum,
    DVE tensor_tensor E*E->bf16 E2 (2x_1p mode) + tensor_scalar copy pass
    with fused fp32 row-sum (4x_2p mode),
    PE ones-matmul per-column sums into PSUM (E at partition 0, E2 at 32).
  - All 8 input DMAs issued up-front (lpool bufs=8) so HBM streams ahead
    of ACT; dummy exp before the loop hoists the ACT table load under DMA.
Host: assemble sums, per-row loss formula over 4096 rows in f64, mean.
"""

import os

import numpy as np
import ml_dtypes

import concourse.bacc as bacc
import concourse.bass as bass
import concourse.tile as tile
from concourse import mybir
from concourse.bass_utils import run_bass_kernel_spmd

B = 4096
N_CORES = 8
ROWS_PER_CORE = B // N_CORES  # 512
P = 128
TILES = ROWS_PER_CORE // P  # 4
CHUNK = 512  # matmul free-dim max (one PSUM bank)
HALF = B // 2  # 2048 cols per half-tile
NHALF = 2 * TILES  # 8 half-tiles per core

TAU_PLUS = 0.1
TEMPERATURE = 0.5
EPS = 1e-8

USE_BF16_IN = bool(int(os.environ.get("KERNEL_BF16_IN", "1")))
# "stt":   single scalar_tensor_tensor (1x rate) — square + fused rowsum in
#          one 2252ns pass; measured faster than any split (accum_out forces
#          1x on HW, so tensor_tensor(2x) + tensor_scalar-accum(1x) loses).
# "split": tensor_tensor square + tensor_scalar accum pass
SQ_MODE = os.environ.get("KERNEL_SQ", "stt")
USE_CSTAT = bool(int(os.environ.get("KERNEL_CSTAT", "1")))  # chunk-stationary colsums
LPOOL_BUFS = int(os.environ.get("KERNEL_LPOOL_BUFS", "8"))
EPOOL_BUFS = int(os.environ.get("KERNEL_EPOOL_BUFS", "4"))
WARM_ACT = bool(int(os.environ.get("KERNEL_WARM_ACT", "1")))

_NC = None
LAST_RESULTS = None  # BassKernelResults of the most recent run (for test harness)

# Work pieces per core: (row_tile, col_start, col_len).
PIECES = [(h // 2, (h % 2) * HALF, HALF) for h in range(NHALF)]
NPIECE = len(PIECES)  # 8


def _build_bass():
    in_dt = mybir.dt.bfloat16 if USE_BF16_IN else mybir.dt.float32
    edt = mybir.dt.bfloat16

    nc = bacc.Bacc(None)
    # Piece-major packed layout: each [128, 2048] piece is contiguous in
    # DRAM, so its dma_start is a linear 512KB read (the row-strided view of
    # a [512, 4096] slab capped out at ~120 GB/s per transfer).
    slab = nc.declare_dram_parameter(
        "slab", [NPIECE, P, HALF], in_dt, isOutput=False
    )
    rowsums = nc.declare_dram_parameter(
        "rowsums", [P, 2 * NPIECE], mybir.dt.float32, isOutput=True
    )
    # Chunk-stationary layout: [128, 64] (E cols 0:32, E2 cols 32:64), where
    # colsum[m*128 + j] = out[j, m]. Row layout: [2, B].
    cshape = [P, 2 * (B // P)] if USE_CSTAT else [2, B]
    colsums = nc.declare_dram_parameter(
        "colsums", cshape, mybir.dt.float32, isOutput=True
    )

    with tile.TileContext(nc) as tc:
        with (
            tc.tile_pool(name="lpool", bufs=LPOOL_BUFS) as lpool,
            tc.tile_pool(name="epool", bufs=EPOOL_BUFS) as epool,
            tc.tile_pool(name="e2pool", bufs=EPOOL_BUFS) as e2pool,
            tc.tile_pool(name="singles", bufs=1) as singles,
            tc.tile_pool(name="psum", bufs=1, space="PSUM") as psum_pool,
        ):
            ones = nc.const_aps.tensor(1.0, (P, 1), mybir.dt.bfloat16)
            rs = singles.tile([P, 2 * NPIECE], mybir.dt.float32)
            junk = singles.tile([P, HALF], edt) if SQ_MODE == "split" else None
            warm = singles.tile([P, 1], mybir.dt.float32)
            # One PSUM bank per accumulator; output [128, 32] each.
            psE = psum_pool.tile([P, B // P], mybir.dt.float32)
            psE2 = psum_pool.tile([P, B // P], mybir.dt.float32)

            if WARM_ACT:
                # Loads the EXP activation table (~1.3us) while input DMAs run.
                nc.scalar.activation(
                    out=warm,
                    in_=ones,
                    func=mybir.ActivationFunctionType.Exp,
                    scale=2.0,
                )

            # Issue every input DMA up-front; lpool has one buf per piece, so
            # nothing gates them and HBM streams at full rate from t=0.
            # Descriptor generation (DIRECT2D) costs ~620ns of ISSUING-engine
            # time per dma_start, so alternate issuance between the two
            # DMA-capable idle engines (sync, gpsimd).
            issuers = [nc.sync, nc.gpsimd]
            ltiles = []
            for i in range(NPIECE):
                ltile = lpool.tile([P, HALF], in_dt, tag="ltile")
                issuers[i % len(issuers)].dma_start(out=ltile, in_=slab[i, :, :])
                ltiles.append(ltile)

            for i, (t, c0, clen) in enumerate(PIECES):
                ltile = ltiles[i]
                etile = epool.tile([P, clen], edt, tag="etile")
                nc.scalar.activation(
                    out=etile,
                    in_=ltile,
                    func=mybir.ActivationFunctionType.Exp,
                    scale=2.0,
                    accum_out=rs[:, i : i + 1],
                )
                e2tile = e2pool.tile([P, clen], edt, tag="e2tile")
                if SQ_MODE == "split":
                    # E2 = E*E on DVE in 2x_1p mode, then a copy pass whose
                    # fused fp32 accumulator (4x_2p mode) yields the row-sum.
                    nc.vector.tensor_tensor(
                        out=e2tile, in0=etile, in1=etile, op=mybir.AluOpType.mult
                    )
                    nc.vector.tensor_scalar(
                        out=junk,
                        in0=e2tile,
                        scalar1=1.0,
                        scalar2=0.0,
                        op0=mybir.AluOpType.mult,
                        op1=mybir.AluOpType.add,
                        accum_out=rs[:, NPIECE + i : NPIECE + i + 1],
                    )
                else:
                    nc.vector.scalar_tensor_tensor(
                        out=e2tile,
                        in0=etile,
                        scalar=1.0,
                        in1=etile,
                        op0=mybir.AluOpType.mult,
                        op1=mybir.AluOpType.mult,
                        accum_out=rs[:, NPIECE + i : NPIECE + i + 1],
                    )

                # PSUM start_tensor_calc zeroes the whole 2KB (partition, bank)
                # zero-region lazily: only the FIRST matmul touching each psum
                # tensor may carry start=True; later writes to still-pending
                # bytes replace (i.e. add to zero), writes to touched bytes
                # accumulate. One start per tensor, ever.
                first = i == 0
                last = i == NPIECE - 1
                for m in range(clen // P):
                    gm = c0 // P + m
                    lsl = slice(m * P, (m + 1) * P)
                    nc.tensor.matmul(
                        psE[:, gm : gm + 1],
                        etile[:, lsl],
                        ones,
                        start=first and m == 0,
                        stop=last and m == clen // P - 1,
                        skip_group_check=True,
                    )
                for m in range(clen // P):
                    gm = c0 // P + m
                    lsl = slice(m * P, (m + 1) * P)
                    nc.tensor.matmul(
                        psE2[:, gm : gm + 1],
                        e2tile[:, lsl],
                        ones,
                        start=first and m == 0,
                        stop=last and m == clen // P - 1,
                        skip_group_check=True,
                    )

            # rowsums is ready at the last accum write; colsums needs the
            # PSUM eviction copies first. Issue the two output DMAs from
            # different engines so their descriptor generation overlaps.
            nc.sync.dma_start(out=rowsums[:, :], in_=rs)
            M = B // P  # 32
            cs = singles.tile([P, 2 * M], mybir.dt.float32)
            nc.vector.tensor_copy(cs[:, 0:M], psE)
            nc.scalar.copy(cs[:, M : 2 * M], psE2)
            nc.gpsimd.dma_start(out=colsums[:, :], in_=cs)
    # Bacc defers register allocation and sync-wait splitting to finalize();
    # run_bass_via_pjrt does not call it, so do it here.
    nc.finalize()
    return nc


def _get_nc():
    global _NC
    if _NC is None:
        _NC = _build_bass()
    return _NC


def kernel(logits: np.ndarray) -> np.ndarray:
    global LAST_RESULTS
    logits = np.ascontiguousarray(np.asarray(logits, dtype=np.float32))
    assert logits.shape == (B, B)

    nc = _get_nc()
    full = logits.astype(ml_dtypes.bfloat16) if USE_BF16_IN else logits
    # Pack each core's slab piece-major ([8, 128, 2048], PIECES order) so
    # every piece is one contiguous 512KB DMA read.
    in_maps = []
    for k in range(N_CORES):
        sl = full[k * ROWS_PER_CORE : (k + 1) * ROWS_PER_CORE, :]
        packed = np.ascontiguousarray(
            sl.reshape(TILES, P, 2, HALF).transpose(0, 2, 1, 3).reshape(
                NPIECE, P, HALF
            )
        )
        in_maps.append({"slab": packed})
    res = run_bass_kernel_spmd(
        nc,
        in_maps,
        core_ids=list(range(N_CORES)),
        trace=bool(int(os.environ.get("KERNEL_TRACE", "0"))),
    )
    LAST_RESULTS = res

    rowsum_E = np.empty(B, dtype=np.float64)
    rowsum_E2 = np.empty(B, dtype=np.float64)
    colsum_E = np.zeros(B, dtype=np.float64)
    colsum_E2 = np.zeros(B, dtype=np.float64)
    for k in range(N_CORES):
        r = res.results[k]
        rs = r["rowsums"].astype(np.float64)  # [128, 2*NPIECE]: piece partials
        sl = slice(k * ROWS_PER_CORE, (k + 1) * ROWS_PER_CORE)
        rsE = np.zeros((P, TILES))
        rsE2 = np.zeros((P, TILES))
        for i, (t, _, _) in enumerate(PIECES):
            rsE[:, t] += rs[:, i]
            rsE2[:, t] += rs[:, NPIECE + i]
        rowsum_E[sl] = rsE.T.reshape(-1)
        rowsum_E2[sl] = rsE2.T.reshape(-1)
        cssum = r["colsums"].astype(np.float64)
        if USE_CSTAT:
            M = B // P
            colsum_E += cssum[:, :M].T.reshape(-1)
            colsum_E2 += cssum[:, M:].T.reshape(-1)
        else:
            colsum_E += cssum[0]
            colsum_E2 += cssum[1]

    d = np.diagonal(logits)
    pos = np.exp(d.astype(np.float64) / TEMPERATURE)
    if USE_BF16_IN:
        # The device sums contain exp of the bf16-rounded diagonal; subtract
        # exactly what the device added.
        dD = d.astype(ml_dtypes.bfloat16).astype(np.float64)
    else:
        dD = d.astype(np.float64)
    posD = np.exp(dD / TEMPERATURE)
    N = 2 * B - 2
    S1 = rowsum_E + colsum_E - 2.0 * posD
    S2 = rowsum_E2 + colsum_E2 - 2.0 * posD * posD
    reweight = S2 * N / S1
    Ng = (-TAU_PLUS * N * pos + reweight) / (1.0 - TAU_PLUS)
    Ng = np.maximum(Ng, N * np.exp(-1.0 / TEMPERATURE))
    loss = -np.log(pos / (pos + Ng + EPS))
    return np.float32(loss.mean())


# revision 16
# speedup vs baseline: 1.1053x; 1.0170x over previous
"""CLIP-style contrastive (HCL) loss for B=4096, f32 logits on 8 trn2 cores.

Math reduction (BETA=1, t=0.5, tau+=0.1):
  - imp == neg, so reweight_neg = sum(neg^2) * N / sum(neg).
  - Row i and row i+B of the 2Bx2B sim matrix hold identical value multisets
    (both are {row_i(L), col_i(L)} minus two copies of L[i,i]), so
    loss[i] == loss[i+B] and the mean over 2B rows == mean over B rows.
  - Everything reduces to row sums + col sums of E = exp(2L) and E2 = exp(4L),
    plus the diagonal of L.

Device work per core (rows k*512..(k+1)*512 of L, cast to bf16 on host and
packed piece-major so every DMA is a contiguous linear read):
  - 8 half-tiles [128, 2048]: ACT exp(2x)->bf16 E with fused fp32 row-sum;
    DVE scalar_tensor_tensor E*E->bf16 E2 with fused fp32 row-sum (last
    piece's square runs on ACT instead -- Square shares the exp_and_friends
    table so no reload -- letting DVE, the drain-limiting engine, finish a
    piece early); PE ones-matmul per-column sums into PSUM.
  - All input DMAs issued up-front from the two DMA-capable idle engines
    (sync, gpsimd); pieces 0/1 split row-wise so the pipeline-head data
    lands across more DMA queues sooner. Dummy exp hoists the ~1.3us ACT
    table load under the DMA wait. Outputs DMA'd straight from PSUM.
Host: assemble sums, per-row loss formula over 4096 rows in f64, mean.
"""

import os

import numpy as np
import ml_dtypes

import concourse.bacc as bacc
import concourse.bass as bass
import concourse.tile as tile
from concourse import mybir
from concourse.bass_utils import run_bass_kernel_spmd

B = 4096
N_CORES = 8
ROWS_PER_CORE = B // N_CORES  # 512
P = 128
TILES = ROWS_PER_CORE // P  # 4
HALF = B // 2  # 2048 cols per half-tile
NHALF = 2 * TILES  # 8 half-tiles per core
M = B // P  # 32 column-chunks

TAU_PLUS = 0.1
TEMPERATURE = 0.5
EPS = 1e-8

USE_BF16_IN = bool(int(os.environ.get("KERNEL_BF16_IN", "1")))
LPOOL_BUFS = int(os.environ.get("KERNEL_LPOOL_BUFS", "8"))
EPOOL_BUFS = int(os.environ.get("KERNEL_EPOOL_BUFS", "4"))
WARM_ACT = bool(int(os.environ.get("KERNEL_WARM_ACT", "1")))
ACT_SQ = bool(int(os.environ.get("KERNEL_ACT_SQ", "1")))  # last square on ACT
DMA_SPLIT = int(os.environ.get("KERNEL_DMA_SPLIT", "2"))  # head pieces split

_NC = None
LAST_RESULTS = None  # BassKernelResults of the most recent run (for test harness)

# Work pieces per core: (row_tile, col_start). Packed input is piece-major.
PIECES = [(h // 2, (h % 2) * HALF) for h in range(NHALF)]
NPIECE = len(PIECES)  # 8


def _build_bass():
    in_dt = mybir.dt.bfloat16 if USE_BF16_IN else mybir.dt.float32
    edt = mybir.dt.bfloat16

    nc = bacc.Bacc(None)
    # Piece-major packed layout: each [128, 2048] piece is contiguous in
    # DRAM, so its dma_start is a linear 512KB read (a row-strided view of
    # a [512, 4096] slab capped out at ~120 GB/s per transfer).
    slab = nc.declare_dram_parameter("slab", [NPIECE, P, HALF], in_dt, isOutput=False)
    rowsE = nc.declare_dram_parameter(
        "rowsE", [P, NPIECE], mybir.dt.float32, isOutput=True
    )
    rowsE2 = nc.declare_dram_parameter(
        "rowsE2", [P, NPIECE], mybir.dt.float32, isOutput=True
    )
    # Chunk-stationary layout [128, 32]: colsum[m*128 + j] = out[j, m].
    colsE = nc.declare_dram_parameter("colsE", [P, M], mybir.dt.float32, isOutput=True)
    colsE2 = nc.declare_dram_parameter(
        "colsE2", [P, M], mybir.dt.float32, isOutput=True
    )

    with tile.TileContext(nc) as tc:
        with (
            tc.tile_pool(name="lpool", bufs=LPOOL_BUFS) as lpool,
            tc.tile_pool(name="epool", bufs=EPOOL_BUFS) as epool,
            tc.tile_pool(name="e2pool", bufs=EPOOL_BUFS) as e2pool,
            tc.tile_pool(name="singles", bufs=1) as singles,
            tc.tile_pool(name="psum", bufs=1, space="PSUM") as psum_pool,
        ):
            ones = nc.const_aps.tensor(1.0, (P, 1), mybir.dt.bfloat16)
            rsE = singles.tile([P, NPIECE], mybir.dt.float32)
            rsE2 = singles.tile([P, NPIECE], mybir.dt.float32)
            warm = singles.tile([P, 1], mybir.dt.float32)
            # One PSUM bank per accumulator; output [128, 32] each.
            psE = psum_pool.tile([P, M], mybir.dt.float32)
            psE2 = psum_pool.tile([P, M], mybir.dt.float32)

            if WARM_ACT:
                # Loads the EXP activation table (~1.3us) while input DMAs run.
                nc.scalar.activation(
                    out=warm,
                    in_=ones,
                    func=mybir.ActivationFunctionType.Exp,
                    scale=2.0,
                )

            # Issue every input DMA up-front; lpool has one buf per piece, so
            # nothing gates them and HBM streams from t=0. Descriptor
            # generation (DIRECT2D) costs ~650ns of issuing-engine time per
            # dma_start, so alternate between the two DMA-capable idle
            # engines. A single dma_start only engages a few of the 16 DMA
            # queues (~135 GB/s), so split the first DMA_SPLIT pieces into
            # row-halves to land the pipeline-head data sooner.
            issuers = [nc.sync, nc.gpsimd]
            nissue = 0

            def issue(out_ap, in_ap):
                nonlocal nissue
                issuers[nissue % len(issuers)].dma_start(out=out_ap, in_=in_ap)
                nissue += 1

            ltiles = []
            for i in range(NPIECE):
                ltile = lpool.tile([P, HALF], in_dt, tag="ltile")
                if i < DMA_SPLIT:
                    issue(ltile[0 : P // 2, :], slab[i, 0 : P // 2, :])
                    issue(ltile[P // 2 : P, :], slab[i, P // 2 : P, :])
                else:
                    issue(ltile, slab[i, :, :])
                ltiles.append(ltile)

            for i, (t, c0) in enumerate(PIECES):
                ltile = ltiles[i]
                etile = epool.tile([P, HALF], edt, tag="etile")
                nc.scalar.activation(
                    out=etile,
                    in_=ltile,
                    func=mybir.ActivationFunctionType.Exp,
                    scale=2.0,
                    accum_out=rsE[:, i : i + 1],
                )
                e2tile = e2pool.tile([P, HALF], edt, tag="e2tile")
                if ACT_SQ and i == NPIECE - 1:
                    # Last square on ACT: DVE (the drain-limiting engine)
                    # skips a piece; Square is in the exp_and_friends table
                    # so there is no table reload.
                    nc.scalar.activation(
                        out=e2tile,
                        in_=etile,
                        func=mybir.ActivationFunctionType.Square,
                        accum_out=rsE2[:, i : i + 1],
                    )
                else:
                    nc.vector.scalar_tensor_tensor(
                        out=e2tile,
                        in0=etile,
                        scalar=1.0,
                        in1=etile,
                        op0=mybir.AluOpType.mult,
                        op1=mybir.AluOpType.mult,
                        accum_out=rsE2[:, i : i + 1],
                    )

                # PSUM start_tensor_calc zeroes the whole 2KB (partition, bank)
                # zero-region lazily: only the FIRST matmul touching each psum
                # tensor may carry start=True; later writes to still-pending
                # bytes replace (i.e. add to zero), writes to touched bytes
                # accumulate. One start per tensor, ever.
                first = i == 0
                last = i == NPIECE - 1
                nmm = HALF // P  # 16
                for m in range(nmm):
                    gm = c0 // P + m
                    lsl = slice(m * P, (m + 1) * P)
                    nc.tensor.matmul(
                        psE[:, gm : gm + 1],
                        etile[:, lsl],
                        ones,
                        start=first and m == 0,
                        stop=last and m == nmm - 1,
                        skip_group_check=True,
                    )
                for m in range(nmm):
                    gm = c0 // P + m
                    lsl = slice(m * P, (m + 1) * P)
                    nc.tensor.matmul(
                        psE2[:, gm : gm + 1],
                        e2tile[:, lsl],
                        ones,
                        start=first and m == 0,
                        stop=last and m == nmm - 1,
                        skip_group_check=True,
                    )

            # Outputs spread over both DMA engines so descriptor generation
            # overlaps; each is issued as soon as its producer chain
            # finishes. PSUM cannot source a DMA, so colsums stage through
            # SBUF via copies on the (by then idle) vector engine.
            csE = singles.tile([P, M], mybir.dt.float32)
            csE2 = singles.tile([P, M], mybir.dt.float32)
            nc.sync.dma_start(out=rowsE[:, :], in_=rsE)
            nc.gpsimd.dma_start(out=rowsE2[:, :], in_=rsE2)
            nc.vector.tensor_copy(csE, psE)
            nc.vector.tensor_copy(csE2, psE2)
            nc.sync.dma_start(out=colsE[:, :], in_=csE)
            nc.gpsimd.dma_start(out=colsE2[:, :], in_=csE2)
    # Bacc defers register allocation and sync-wait splitting to finalize();
    # run_bass_via_pjrt does not call it, so do it here.
    nc.finalize()
    return nc


def _get_nc():
    global _NC
    if _NC is None:
        _NC = _build_bass()
    return _NC


def kernel(logits: np.ndarray) -> np.ndarray:
    global LAST_RESULTS
    logits = np.ascontiguousarray(np.asarray(logits, dtype=np.float32))
    assert logits.shape == (B, B)

    nc = _get_nc()
    full = logits.astype(ml_dtypes.bfloat16) if USE_BF16_IN else logits
    # Pack each core's slab piece-major ([8, 128, 2048], PIECES order) so
    # every piece is one contiguous DMA read.
    in_maps = []
    for k in range(N_CORES):
        sl = full[k * ROWS_PER_CORE : (k + 1) * ROWS_PER_CORE, :]
        packed = np.ascontiguousarray(
            sl.reshape(TILES, P, 2, HALF).transpose(0, 2, 1, 3).reshape(NPIECE, P, HALF)
        )
        in_maps.append({"slab": packed})
    res = run_bass_kernel_spmd(
        nc,
        in_maps,
        core_ids=list(range(N_CORES)),
        trace=bool(int(os.environ.get("KERNEL_TRACE", "0"))),
    )
    LAST_RESULTS = res

    rowsum_E = np.empty(B, dtype=np.float64)
    rowsum_E2 = np.empty(B, dtype=np.float64)
    colsum_E = np.zeros(B, dtype=np.float64)
    colsum_E2 = np.zeros(B, dtype=np.float64)
    for k in range(N_CORES):
        r = res.results[k]
        rse = r["rowsE"].astype(np.float64)  # [128, NPIECE] piece partials
        rse2 = r["rowsE2"].astype(np.float64)
        sl = slice(k * ROWS_PER_CORE, (k + 1) * ROWS_PER_CORE)
        rsE = np.zeros((P, TILES))
        rsE2 = np.zeros((P, TILES))
        for i, (t, _) in enumerate(PIECES):
            rsE[:, t] += rse[:, i]
            rsE2[:, t] += rse2[:, i]
        rowsum_E[sl] = rsE.T.reshape(-1)
        rowsum_E2[sl] = rsE2.T.reshape(-1)
        # Chunk-stationary: colsum[m*128 + j] = cs[j, m]
        colsum_E += r["colsE"].astype(np.float64).T.reshape(-1)
        colsum_E2 += r["colsE2"].astype(np.float64).T.reshape(-1)

    d = np.diagonal(logits)
    pos = np.exp(d.astype(np.float64) / TEMPERATURE)
    if USE_BF16_IN:
        # The device sums contain exp of the bf16-rounded diagonal; subtract
        # exactly what the device added.
        dD = d.astype(ml_dtypes.bfloat16).astype(np.float64)
    else:
        dD = d.astype(np.float64)
    posD = np.exp(dD / TEMPERATURE)
    N = 2 * B - 2
    S1 = rowsum_E + colsum_E - 2.0 * posD
    S2 = rowsum_E2 + colsum_E2 - 2.0 * posD * posD
    reweight = S2 * N / S1
    Ng = (-TAU_PLUS * N * pos + reweight) / (1.0 - TAU_PLUS)
    Ng = np.maximum(Ng, N * np.exp(-1.0 / TEMPERATURE))
    loss = -np.log(pos / (pos + Ng + EPS))
    return np.float32(loss.mean())
